# revision 3
# baseline (speedup 1.0000x reference)
"""Trainium2 Bass kernel for a dense-MoE FFN layer (top-2 routing).

Expert-parallel over 8 NeuronCores: core e owns expert e (W1[e], W2[e]).
Every core:
  - computes fp32 router logits for all tokens (replicated router),
    derives its own expert's per-token top-2 softmax weight on device,
  - runs the dense expert FFN in bf16 (fp32 accumulate in PSUM),
  - scales by the router weight, writes a partial sum [N, D],
  - ReduceScatter(+) over the 8 cores -> each core holds the summed
    MoE output for a distinct 512-token slice,
  - adds the residual and applies LayerNorm on that slice.
The host concatenates the 8 slices into the full [B, S, D] output.
"""

import numpy as np
import ml_dtypes

B, S, D, F, E = 2, 2048, 1024, 4096, 8
N = B * S              # 4096 tokens
NC = 8                 # cores
TSLICE = N // NC       # 512 tokens output slice per core
TB = 512               # token block for the matmul pipeline
NB = N // TB           # 8 blocks
ND = D // 128          # 8 d-tiles
NF = F // 128          # 32 f-tiles
NT = N // 128          # 32 token tiles
LN_EPS = 1e-5

BF16 = ml_dtypes.bfloat16

_CACHE = {}


def _build_nc():
    import concourse.bacc as bacc
    import concourse.mybir as mybir
    import concourse.tile as tile

    dt = mybir.dt
    f32, bf16 = dt.float32, dt.bfloat16
    Alu = mybir.AluOpType
    Act = mybir.ActivationFunctionType
    AX = mybir.AxisListType.X

    nc = bacc.Bacc(num_devices=NC)

    xtf = nc.dram_tensor("xtf", [D, N], f32, kind="ExternalInput")
    xtb = nc.dram_tensor("xtb", [D, N], bf16, kind="ExternalInput")
    w1t = nc.dram_tensor("w1t", [D, F], bf16, kind="ExternalInput")
    w2t = nc.dram_tensor("w2t", [F, D], bf16, kind="ExternalInput")
    b1c = nc.dram_tensor("b1c", [128, NF], f32, kind="ExternalInput")
    b2r = nc.dram_tensor("b2r", [128, D], f32, kind="ExternalInput")
    wrt = nc.dram_tensor("wrt", [D, E], f32, kind="ExternalInput")
    brr = nc.dram_tensor("brr", [128, E], f32, kind="ExternalInput")
    xres = nc.dram_tensor("xres", [TSLICE, D], f32, kind="ExternalInput")
    gmr = nc.dram_tensor("gmr", [128, D], f32, kind="ExternalInput")
    btr = nc.dram_tensor("btr", [128, D], f32, kind="ExternalInput")
    out = nc.dram_tensor("out", [TSLICE, D], f32, kind="ExternalOutput")

    xtf_r = xtf.ap().rearrange("(a p) n -> a p n", p=128)
    xtb_r = xtb.ap().rearrange("(a p) n -> a p n", p=128)
    w1t_r = w1t.ap().rearrange("(a p) f -> a p f", p=128)
    w2t_r = w2t.ap().rearrange("(a p) d -> a p d", p=128)
    wrt_r = wrt.ap().rearrange("(a p) e -> a p e", p=128)
    xres_r = xres.ap().rearrange("(a p) d -> a p d", p=128)
    out_r = out.ap().rearrange("(a p) d -> a p d", p=128)

    with tile.TileContext(nc) as tc:
        with (
            tc.tile_pool(name="wts", bufs=1) as wts,
            tc.tile_pool(name="xs", bufs=1) as xs_pool,
            tc.tile_pool(name="stage", bufs=4) as stage_pool,
            tc.tile_pool(name="psr", bufs=2, space="PSUM") as psum_r,
            tc.tile_pool(name="psh", bufs=2, space="PSUM") as psum_h,
            tc.tile_pool(name="pso", bufs=4, space="PSUM") as psum_o,
            tc.tile_pool(name="dram", bufs=1, space="DRAM") as dram,
        ):
            # --- persistent small tensors ---
            wrt_sb = []
            for d0 in range(ND):
                t = wts.tile([128, E], f32, name=f"wrt{d0}", tag=f"wrt{d0}")
                nc.sync.dma_start(t[:], wrt_r[d0])
                wrt_sb.append(t)
            brr_sb = wts.tile([128, E], f32, name="brr_sb")
            nc.sync.dma_start(brr_sb[:], brr[:])
            b1_sb = wts.tile([128, NF], f32, name="b1_sb")
            nc.sync.dma_start(b1_sb[:], b1c[:])
            b2_sb = wts.tile([128, D], f32, name="b2_sb")
            nc.sync.dma_start(b2_sb[:], b2r[:])
            gm_sb = wts.tile([128, D], f32, name="gm_sb")
            nc.sync.dma_start(gm_sb[:], gmr[:])
            bt_sb = wts.tile([128, D], f32, name="bt_sb")
            nc.sync.dma_start(bt_sb[:], btr[:])
            # per-token router weight for this core's expert, [128, NT]
            w_all = wts.tile([128, NT], f32, name="w_all")
            eps_sb = wts.tile([128, 1], f32, name="eps_sb")
            nc.vector.memset(eps_sb[:], LN_EPS)

            # --- expert weights (persistent, stream in behind the router) ---
            w1_sb = []
            for d0 in range(ND):
                t = wts.tile([128, F], bf16, name=f"w1_{d0}", tag=f"w1_{d0}")
                nc.sync.dma_start(t[:], w1t_r[d0])
                w1_sb.append(t)
            w2_sb = []
            for f0 in range(NF):
                t = wts.tile([128, D], bf16, name=f"w2_{f0}", tag=f"w2_{f0}")
                nc.sync.dma_start(t[:], w2t_r[f0])
                w2_sb.append(t)

            # --- router phase: fp32 logits -> top-2 weight for own expert ---
            with (
                tc.tile_pool(name="xtfp", bufs=2) as xtf_pool,
                tc.tile_pool(name="rtmp", bufs=4) as rtmp,
            ):
                for blk in range(NB):
                    xf = []
                    for d0 in range(ND):
                        t = xtf_pool.tile([128, TB], f32, name=f"xf{d0}", tag=f"xf{d0}")
                        nc.sync.dma_start(t[:], xtf_r[d0][:, blk * TB:(blk + 1) * TB])
                        xf.append(t)
                    for tt in range(TB // 128):
                        tok = blk * (TB // 128) + tt
                        ps = psum_r.tile([128, E], f32, name="ps_r", tag="ps_r")
                        for d0 in range(ND):
                            nc.tensor.matmul(
                                ps[:],
                                lhsT=xf[d0][:, tt * 128:(tt + 1) * 128],
                                rhs=wrt_sb[d0][:],
                                start=(d0 == 0),
                                stop=(d0 == ND - 1),
                            )
                        lg = rtmp.tile([128, E], f32, name="lg", tag="lg")
                        nc.vector.tensor_tensor(lg[:], ps[:], brr_sb[:], op=Alu.add)
                        m1 = rtmp.tile([128, 1], f32, name="m1", tag="m1")
                        nc.vector.reduce_max(m1[:], lg[:], axis=AX)
                        eq = rtmp.tile([128, E], f32, name="eq", tag="eq")
                        nc.vector.tensor_scalar(
                            eq[:], lg[:], m1[:], None, op0=Alu.is_equal
                        )
                        msk = rtmp.tile([128, E], f32, name="msk", tag="msk")
                        nc.vector.scalar_tensor_tensor(
                            msk[:], in0=eq[:], scalar=-1e30, in1=lg[:],
                            op0=Alu.mult, op1=Alu.add,
                        )
                        m2 = rtmp.tile([128, 1], f32, name="m2", tag="m2")
                        nc.vector.reduce_max(m2[:], msk[:], axis=AX)
                        my = lg[:, 0:1]
                        d1 = rtmp.tile([128, 1], f32, name="d1", tag="d1")
                        nc.vector.tensor_tensor(d1[:], my, m2[:], op=Alu.subtract)
                        d2 = rtmp.tile([128, 1], f32, name="d2", tag="d2")
                        nc.vector.tensor_tensor(d2[:], my, m1[:], op=Alu.subtract)
                        s1 = rtmp.tile([128, 1], f32, name="s1", tag="s1")
                        nc.scalar.activation(s1[:], d1[:], Act.Sigmoid)
                        s2 = rtmp.tile([128, 1], f32, name="s2", tag="s2")
                        nc.scalar.activation(s2[:], d2[:], Act.Sigmoid)
                        e1 = rtmp.tile([128, 1], f32, name="e1", tag="e1")
                        nc.vector.tensor_tensor(e1[:], my, m1[:], op=Alu.is_equal)
                        e2 = rtmp.tile([128, 1], f32, name="e2", tag="e2")
                        nc.vector.tensor_tensor(e2[:], my, m2[:], op=Alu.is_equal)
                        t1 = rtmp.tile([128, 1], f32, name="t1", tag="t1")
                        nc.vector.tensor_tensor(t1[:], e1[:], s1[:], op=Alu.mult)
                        t2 = rtmp.tile([128, 1], f32, name="t2", tag="t2")
                        nc.vector.tensor_tensor(t2[:], e2[:], s2[:], op=Alu.mult)
                        nc.vector.tensor_tensor(
                            w_all[:, tok:tok + 1], t1[:], t2[:], op=Alu.add
                        )

            partial = dram.tile([N, D], f32, name="partial")
            partial_r = partial.rearrange("(t p) d -> t p d", p=128)

            # --- expert FFN blocks ---
            with tc.tile_pool(name="htp", bufs=1) as ht_pool:
                for blk in range(NB):
                    xb = []
                    for d0 in range(ND):
                        t = xs_pool.tile([128, TB], bf16, name=f"xb{d0}", tag=f"xb{d0}")
                        nc.sync.dma_start(t[:], xtb_r[d0][:, blk * TB:(blk + 1) * TB])
                        xb.append(t)
                    # h^T = relu(W1 x^T + b1), produced as 32 [128f, TB] bf16 tiles
                    ht = []
                    for f0 in range(NF):
                        hp = psum_h.tile([128, TB], f32, name="hp", tag="hp")
                        for d0 in range(ND):
                            nc.tensor.matmul(
                                hp[:],
                                lhsT=w1_sb[d0][:, f0 * 128:(f0 + 1) * 128],
                                rhs=xb[d0][:],
                                start=(d0 == 0),
                                stop=(d0 == ND - 1),
                            )
                        hs = ht_pool.tile([128, TB], bf16, name=f"ht{f0}", tag=f"ht{f0}")
                        nc.scalar.activation(
                            hs[:], hp[:], Act.Relu, bias=b1_sb[:, f0:f0 + 1]
                        )
                        ht.append(hs)
                    # o = h W2^T + b2, weighted by router w, to partial DRAM
                    for ts in range(TB // 128):
                        tok = blk * (TB // 128) + ts
                        op0 = psum_o.tile([128, 512], f32, name="op0", tag="op")
                        op1 = psum_o.tile([128, 512], f32, name="op1", tag="op")
                        for f0 in range(NF):
                            nc.tensor.matmul(
                                op0[:],
                                lhsT=ht[f0][:, ts * 128:(ts + 1) * 128],
                                rhs=w2_sb[f0][:, 0:512],
                                start=(f0 == 0),
                                stop=(f0 == NF - 1),
                            )
                            nc.tensor.matmul(
                                op1[:],
                                lhsT=ht[f0][:, ts * 128:(ts + 1) * 128],
                                rhs=w2_sb[f0][:, 512:1024],
                                start=(f0 == 0),
                                stop=(f0 == NF - 1),
                            )
                        for dn, op_ in enumerate((op0, op1)):
                            st = stage_pool.tile([128, 512], f32, name="st", tag="st")
                            nc.vector.tensor_tensor(
                                st[:], op_[:], b2_sb[:, dn * 512:(dn + 1) * 512],
                                op=Alu.add,
                            )
                            nc.vector.tensor_scalar_mul(
                                st[:], st[:], w_all[:, tok:tok + 1]
                            )
                            nc.sync.dma_start(
                                partial_r[tok][:, dn * 512:(dn + 1) * 512], st[:]
                            )

                rs_out = dram.tile([TSLICE, D], f32, name="rs_out")
                nc.gpsimd.collective_compute(
                    "ReduceScatter",
                    Alu.add,
                    replica_groups=[list(range(NC))],
                    ins=[partial.opt()],
                    outs=[rs_out.opt()],
                )
            rs_r = rs_out.rearrange("(t p) d -> t p d", p=128)

            # --- residual + LayerNorm on own 512-token slice ---
            with tc.tile_pool(name="ln", bufs=1) as ln_pool:
                for i in range(TSLICE // 128):
                    rs_sb = ln_pool.tile([128, D], f32, name="rs_sb", tag="rs")
                    nc.sync.dma_start(rs_sb[:], rs_r[i])
                    xr = ln_pool.tile([128, D], f32, name="xr", tag="xr")
                    nc.sync.dma_start(xr[:], xres_r[i])
                    nc.vector.tensor_tensor(rs_sb[:], rs_sb[:], xr[:], op=Alu.add)
                    mu = ln_pool.tile([128, 1], f32, name="mu", tag="mu")
                    nc.vector.reduce_sum(mu[:], rs_sb[:], axis=AX)
                    nc.vector.tensor_scalar_mul(mu[:], mu[:], 1.0 / D)
                    xc = ln_pool.tile([128, D], f32, name="xc", tag="xc")
                    nc.vector.tensor_scalar_sub(xc[:], rs_sb[:], mu[:])
                    sq = ln_pool.tile([128, D], f32, name="sq", tag="sq")
                    var = ln_pool.tile([128, 1], f32, name="var", tag="var")
                    nc.scalar.activation(sq[:], xc[:], Act.Square, accum_out=var[:])
                    std = ln_pool.tile([128, 1], f32, name="std", tag="std")
                    nc.scalar.activation(
                        std[:], var[:], Act.Sqrt, scale=1.0 / D, bias=eps_sb[:]
                    )
                    rstd = ln_pool.tile([128, 1], f32, name="rstd", tag="rstd")
                    nc.vector.reciprocal(rstd[:], std[:])
                    o_sb = ln_pool.tile([128, D], f32, name="o_sb", tag="o")
                    nc.vector.scalar_tensor_tensor(
                        o_sb[:], in0=xc[:], scalar=rstd[:], in1=gm_sb[:],
                        op0=Alu.mult, op1=Alu.mult,
                    )
                    nc.vector.tensor_tensor(o_sb[:], o_sb[:], bt_sb[:], op=Alu.add)
                    nc.sync.dma_start(out_r[i], o_sb[:])

    nc.finalize()
    return nc


def _get_nc():
    if "nc" not in _CACHE:
        _CACHE["nc"] = _build_nc()
    return _CACHE["nc"]


def _build_in_maps(tgt, Wr, br, W1, b1, W2, b2, gamma, beta):
    f32 = np.float32
    tgt = np.asarray(tgt, dtype=f32)
    Wr = np.asarray(Wr, dtype=f32)
    br = np.asarray(br, dtype=f32)
    W1 = np.asarray(W1, dtype=f32)
    b1 = np.asarray(b1, dtype=f32)
    W2 = np.asarray(W2, dtype=f32)
    b2 = np.asarray(b2, dtype=f32)
    gamma = np.asarray(gamma, dtype=f32)
    beta = np.asarray(beta, dtype=f32)

    x = np.ascontiguousarray(tgt.reshape(N, D))
    xt = np.ascontiguousarray(x.T)                      # [D, N] fp32
    xtb = xt.astype(BF16)
    gmr = np.ascontiguousarray(np.broadcast_to(gamma, (128, D)))
    btr = np.ascontiguousarray(np.broadcast_to(beta, (128, D)))

    in_maps = []
    for e in range(NC):
        perm = [e] + [i for i in range(E) if i != e]
        in_maps.append({
            "xtf": xt,
            "xtb": xtb,
            "w1t": np.ascontiguousarray(W1[e].T).astype(BF16),   # [D, F]
            "w2t": np.ascontiguousarray(W2[e].T).astype(BF16),   # [F, D]
            "b1c": np.ascontiguousarray(b1[e].reshape(NF, 128).T),  # [128, NF]
            "b2r": np.ascontiguousarray(np.broadcast_to(b2[e], (128, D))),
            "wrt": np.ascontiguousarray(Wr[perm].T),             # [D, E]
            "brr": np.ascontiguousarray(np.broadcast_to(br[perm], (128, E))),
            "xres": np.ascontiguousarray(x[e * TSLICE:(e + 1) * TSLICE]),
            "gmr": gmr,
            "btr": btr,
        })
    return in_maps


def _assemble(results):
    out = np.concatenate([results[r]["out"] for r in range(NC)], axis=0)
    return np.ascontiguousarray(out.reshape(B, S, D).astype(np.float32))


def _get_runner():
    """Build (once) a cached jitted SPMD executor mirroring
    concourse.bass2jax.run_bass_via_pjrt, so repeat kernel() calls skip
    recompilation."""
    if "runner" in _CACHE:
        return _CACHE["runner"]

    import jax
    from jax.sharding import Mesh, PartitionSpec
    from jax.experimental.shard_map import shard_map
    import concourse.mybir as mybir
    from concourse import bass2jax

    nc = _get_nc()
    bass2jax.install_neuronx_cc_hook()

    partition_name = nc.partition_id_tensor.name if nc.partition_id_tensor else None
    in_names, out_names, out_avals, zero_outs = [], [], [], []
    for alloc in nc.m.functions[0].allocations:
        if not isinstance(alloc, mybir.MemoryLocationSet):
            continue
        name = alloc.memorylocations[0].name
        if alloc.kind == "ExternalInput":
            if name != partition_name:
                in_names.append(name)
        elif alloc.kind == "ExternalOutput":
            shape = tuple(alloc.tensor_shape)
            dtype = mybir.dt.np(alloc.dtype)
            out_names.append(name)
            out_avals.append(jax.core.ShapedArray(shape, dtype))
            zero_outs.append(np.zeros(shape, dtype))
    n_params = len(in_names)
    n_outs = len(out_avals)
    all_in_names = list(in_names) + list(out_names)
    if partition_name is not None:
        all_in_names.append(partition_name)

    donate = tuple(range(n_params, n_params + n_outs))

    def _body(*args):
        operands = list(args)
        if partition_name is not None:
            operands.append(bass2jax.partition_id_tensor())
        outs = bass2jax._bass_exec_p.bind(
            *operands,
            out_avals=tuple(out_avals),
            in_names=tuple(all_in_names),
            out_names=tuple(out_names),
            lowering_input_output_aliases=(),
            sim_require_finite=True,
            sim_require_nnan=True,
            nc=nc,
        )
        return tuple(outs)

    devices = jax.devices()[:NC]
    mesh = Mesh(np.asarray(devices), ("core",))
    in_specs = (PartitionSpec("core"),) * (n_params + n_outs)
    out_specs = (PartitionSpec("core"),) * n_outs
    sharded = jax.jit(
        shard_map(
            _body, mesh=mesh, in_specs=in_specs, out_specs=out_specs,
            check_rep=False,
        ),
        donate_argnums=donate,
        keep_unused=True,
    )

    def run(in_maps):
        per_core = [[np.asarray(m[name]) for name in in_names] for m in in_maps]
        concat_in = [
            np.concatenate([per_core[c][i] for c in range(NC)], axis=0)
            for i in range(n_params)
        ]
        concat_zeros = [
            np.zeros((NC * z.shape[0], *z.shape[1:]), z.dtype) for z in zero_outs
        ]
        out_arrs = sharded(*concat_in, *concat_zeros)
        return [
            {
                name: np.asarray(out_arrs[i]).reshape(NC, *out_avals[i].shape)[c]
                for i, name in enumerate(out_names)
            }
            for c in range(NC)
        ]

    _CACHE["runner"] = run
    return run


def kernel(tgt, Wr, br, W1, b1, W2, b2, gamma, beta):
    in_maps = _build_in_maps(tgt, Wr, br, W1, b1, W2, b2, gamma, beta)
    try:
        run = _get_runner()
        results = run(in_maps)
    except Exception:
        # Fallback: the well-tested (but recompiling) library path.
        from concourse.bass_utils import run_bass_kernel_spmd
        res = run_bass_kernel_spmd(
            _get_nc(), in_maps, core_ids=list(range(NC))
        )
        results = res.results
    return _assemble(results)
